# revision 16
# baseline (speedup 1.0000x reference)
"""Trainium2 Bass kernel for nn_Conv2DLinalgRMSNorm.

Math: out = RMSNormEps(x @ (sum_l conv_w[l])^T / 20) * norm_w
  where RMSNormEps(v) = v / sqrt(sum_h v^2 + eps*H) * sqrt(H)

Strategy (8 NeuronCores, no cross-core collectives), all GEMM I/O in bf16:
  Host prep (layout/dtype only): conv_w and x^T are cast to bf16; x is
  pre-transposed to [h_in, tok] so L2 needs no on-chip transposes.
  Launch 1 (weight prep, sharded over 128-row output-channel slices):
    core i reads conv_w[:, i*128:(i+1)*128, :] in bf16 (5.2 MB, 2 KiB DMA
    lines on 2 queues), accumulates 20 layers via two running-sum chains
    (evens on DVE, odds on GpSimd), PE transpose-accumulates the two
    chains per 128-block and writes its [1024, 128] slice of W_sum^T bf16.
  Launch 2 (token-parallel GEMM + norm):
    core i loads full W^T (2 MB) + its x^T token slice (2 MB), runs the
    [1024 tok x 1024 x 1024] GEMM with h_in-chunk-outer ordering over 4
    concurrent PSUM tiles (all 8 banks) so the PE streams while DMA fills,
    then fuses LinalgRMSNorm on ACT/DVE.  The 1/20 scaling folds into the
    rsqrt bias: out = y * 32 * rsqrt(sum y^2 + 400*eps*H) * norm_w.
"""
import numpy as np
import ml_dtypes

import concourse.bass as bass
import concourse.mybir as mybir
from concourse.tile import TileContext
from concourse import bass_utils

dt = mybir.dt
BF16 = ml_dtypes.bfloat16
P = 128
H = 1024
NL = 20
B, S = 2, 4096
TOK = B * S            # 8192
NCORES = 8
TPC = TOK // NCORES    # 1024 tokens per core
NCH = H // P           # 8 h_in chunks of 128
NT = TPC // P          # 8 token tiles per core
EPS = 1e-6
SSQ_BIAS = float(NL * NL * EPS * H)   # 0.4096

_ctr = [0]


def _legalize_waits(nc):
    """This walrus build accepts 1 sync wait per instruction (2 on
    EventSemaphore); split excess waits into standalone waits."""
    def fix_block(blk):
        insts = list(blk.instructions)
        out = []
        changed = False
        for inst in insts:
            si = inst.sync_info
            waits = list(si.on_wait) if si and si.on_wait else []
            cap = 2 if isinstance(inst, mybir.InstEventSemaphore) else 1
            if len(waits) > cap:
                changed = True
                keep = waits[:cap]
                extra = waits[cap:]
                for i in range(0, len(extra), 2):
                    chunk = extra[i:i + 2]
                    _ctr[0] += 1
                    ev = mybir.InstEventSemaphore(
                        name=f"I-waitfix-{_ctr[0]}",
                        engine=inst.engine,
                        ins=[],
                        outs=[],
                        sync_info=mybir.SyncInfo(on_wait=chunk, on_update=[]),
                    )
                    out.append(ev)
                si.on_wait = keep
            out.append(inst)
        if changed:
            blk.instructions = out
        for sub in getattr(blk, "blocks", None) or []:
            fix_block(sub)

    for fn in nc.m.functions:
        for blk in fn.blocks:
            fix_block(blk)


def _make_identity(nc, identity):
    nc.gpsimd.memset(identity, 0.0)
    nc.gpsimd.affine_select(
        out=identity,
        in_=identity,
        compare_op=mybir.AluOpType.not_equal,
        fill=1.0,
        base=0,
        pattern=[[-1, identity.shape[0]]],
        channel_multiplier=1,
    )


def build_l1():
    """Weight prep: conv slice [20, 128, 1024] bf16 -> piece [128, 1024] bf16.

    Pure DMA + DVE: per-layer contiguous loads (2 KiB lines) over three DMA
    queues feed two all-bf16 parity running-sum chains on DVE (2-byte
    operands hit the DVE 2x mode); a final add merges them and the piece
    streams out untransposed with 2 KiB lines.  No PE, no PSUM.
    """
    nc = bass.Bass('TRN2', target_bir_lowering=False, debug=False)
    cw = nc.dram_tensor("cw", [NL, P, H], dt.bfloat16, kind="ExternalInput")
    # untransposed piece [128 h_out rows, 1024 h_in] -- 2 KiB DMA lines;
    # the host transposes pieces for free when assembling L2's W^T
    wtp = nc.dram_tensor("wtp", [P, H], dt.bfloat16, kind="ExternalOutput")
    with TileContext(nc) as tc:
        with (
            tc.tile_pool(name="ld", bufs=1) as ld,
            tc.tile_pool(name="acc", bufs=1) as accp,
        ):
            t = ld.tile([P, NL, H], dt.bfloat16, tag="t")
            cwr = cw.rearrange("l p h -> p l h")
            # evens on sync, odds on scalar, last four on the slower
            # gpsimd SWDGE queue
            for k in range(8):
                nc.sync.dma_start(t[:, 2 * k, :], cwr[:, 2 * k, :])
                nc.scalar.dma_start(t[:, 2 * k + 1, :], cwr[:, 2 * k + 1, :])
            for l in range(16, NL):
                nc.gpsimd.dma_start(t[:, l, :], cwr[:, l, :])

            # all-bf16 parity running sums, both on DVE (2x 16-bit mode)
            se = [accp.tile([P, H], dt.bfloat16, tag=f"se{i}", name=f"se{i}") for i in range(2)]
            so = [accp.tile([P, H], dt.bfloat16, tag=f"so{i}", name=f"so{i}") for i in range(2)]
            nc.vector.tensor_add(se[0][:], t[:, 0, :], t[:, 2, :])
            nc.vector.tensor_add(so[0][:], t[:, 1, :], t[:, 3, :])
            ce = co = 0
            for k in range(2, NL // 2):
                nc.vector.tensor_add(se[1 - ce][:], se[ce][:], t[:, 2 * k, :])
                ce = 1 - ce
                nc.vector.tensor_add(so[1 - co][:], so[co][:], t[:, 2 * k + 1, :])
                co = 1 - co

            wb = accp.tile([P, H], dt.bfloat16, tag="wb")
            nc.vector.tensor_add(wb[:], se[ce][:], so[co][:])
            nc.sync.dma_start(wtp[:, :], wb[:])
    _legalize_waits(nc)
    return nc


def build_l2():
    """Token-shard GEMM + LinalgRMSNorm, all-bf16 I/O, no on-chip transposes.

    x^T slice [1024 (h_in), 1024 (tok)] and W^T [1024, 1024] stream in as 8
    h_in chunks each on separate queues; the GEMM runs h_in-chunk-outer over
    4 concurrent [128, 1024] PSUM tiles (2 groups of 4 token tiles) so the
    PE consumes chunks as they land.  Norm fuses on ACT/DVE per token tile.
    """
    nc = bass.Bass('TRN2', target_bir_lowering=False, debug=False)
    xt = nc.dram_tensor("xt", [H, TPC], dt.bfloat16, kind="ExternalInput")
    wt = nc.dram_tensor("wt", [H, H], dt.bfloat16, kind="ExternalInput")
    nw = nc.dram_tensor("nw", [H], dt.float32, kind="ExternalInput")
    y = nc.dram_tensor("y", [TPC, H], dt.bfloat16, kind="ExternalOutput")
    NG = 2               # psum groups
    GT = NT // NG        # 4 token tiles per group
    with TileContext(nc) as tc:
        with (
            tc.tile_pool(name="w", bufs=1) as wp,
            tc.tile_pool(name="sq", bufs=2) as sqp,
            tc.tile_pool(name="yout", bufs=3) as yp,
            tc.tile_pool(name="stat", bufs=4) as stat,
            tc.tile_pool(name="psum", bufs=1, space="PSUM") as psum,
        ):
            xt_sb = wp.tile([P, NCH, TPC], dt.bfloat16, tag="xt_sb")
            wt_sb = wp.tile([P, NCH, H], dt.bfloat16, tag="wt_sb")
            xt_r = xt.rearrange("(c p) t -> p c t", p=P)
            wt_r = wt.rearrange("(c p) o -> p c o", p=P)
            # identity first so the PE warm-up unblocks early; gpsimd DMA
            # issues follow it in that engine's program order
            ident = wp.tile([P, P], dt.float32, tag="ident")
            _make_identity(nc, ident[:])
            # 3-queue fill ordered by first-use: wt chunks 0-5 on scalar,
            # xt 0-3 then wt 6-7 on sync, xt 4-7 on gpsimd
            for hc in range(6):
                nc.scalar.dma_start(wt_sb[:, hc, :], wt_r[:, hc, :])
            for hc in range(4):
                nc.sync.dma_start(xt_sb[:, hc, :], xt_r[:, hc, :])
            for hc in range(6, NCH):
                nc.sync.dma_start(wt_sb[:, hc, :], wt_r[:, hc, :])
            for hc in range(4, NCH):
                nc.gpsimd.dma_start(xt_sb[:, hc, :], xt_r[:, hc, :])
            nwb = wp.tile([P, H], dt.bfloat16, tag="nwb")
            nc.gpsimd.dma_start(nwb[:], nw[None, :].partition_broadcast(P))

            pts = [psum.tile([P, H], dt.float32, tag=f"pt{i}", name=f"pt{i}") for i in range(GT)]
            # PE warm-up into a psum half that hc=0 later resets (start=True)
            for _ in range(12):
                nc.tensor.matmul(pts[0][:, bass.ds(0, P)], ident[:], ident[:],
                                 is_transpose=True, start=True, stop=True)

            def norm_out(tt, pt):
                # ACT copy releases the psum tile fast; the norm then runs
                # entirely from SBUF in bf16 (DVE 2x mode for the STT)
                ptc = sqp.tile([P, H], dt.bfloat16, tag="ptc", name="ptc")
                nc.scalar.activation(
                    ptc[:], pt[:], mybir.ActivationFunctionType.Copy,
                )
                sq = sqp.tile([P, H], dt.bfloat16, tag="sq", name="sq")
                v = stat.tile([P, 1], dt.float32, tag="v", name="v")
                nc.scalar.activation(
                    sq[:], ptc[:], mybir.ActivationFunctionType.Square,
                    accum_out=v[:],
                )
                vb = stat.tile([P, 1], dt.float32, tag="vb", name="vb")
                nc.vector.tensor_scalar(
                    vb[:], v[:], SSQ_BIAS, None, mybir.AluOpType.add,
                )
                rv = stat.tile([P, 1], dt.float32, tag="rv", name="rv")
                nc.vector.reciprocal(rv[:], vb[:])
                s = stat.tile([P, 1], dt.float32, tag="s", name="s")
                nc.scalar.activation(
                    s[:], rv[:], mybir.ActivationFunctionType.Sqrt,
                    scale=float(H),
                )
                ysb = yp.tile([P, H], dt.bfloat16, tag="ysb", name="ysb")
                nc.vector.scalar_tensor_tensor(
                    ysb[:], ptc[:], s[:], nwb[:],
                    op0=mybir.AluOpType.mult, op1=mybir.AluOpType.mult,
                )
                nc.sync.dma_start(y[bass.ds(tt * P, P), :], ysb[:])

            # group 0 (tiles 0-3): hc-outer waves track the DMA fill
            for hc in range(NCH):
                for i in range(GT):
                    lhs = xt_sb[:, hc, bass.ds(i * P, P)]
                    for oh in range(2):
                        osl = bass.ds(oh * 512, 512)
                        nc.tensor.matmul(
                            pts[i][:, osl], lhs, wt_sb[:, hc, osl],
                            start=(hc == 0), stop=(hc == NCH - 1),
                        )
            for i in range(GT):
                norm_out(i, pts[i])
            # tiles 4-7 sequential on slots 0-3: each slot's psum is
            # released ~1us after its stop by the ACT copy
            for i in range(GT):
                tt = GT + i
                for hc in range(NCH):
                    lhs = xt_sb[:, hc, bass.ds(tt * P, P)]
                    for oh in range(2):
                        osl = bass.ds(oh * 512, 512)
                        nc.tensor.matmul(
                            pts[i][:, osl], lhs, wt_sb[:, hc, osl],
                            start=(hc == 0), stop=(hc == NCH - 1),
                        )
                norm_out(tt, pts[i])
    _legalize_waits(nc)
    return nc


_CACHE = {}


def _get(name, builder):
    if name not in _CACHE:
        _CACHE[name] = builder()
    return _CACHE[name]


def prep_l1_inputs(conv_w):
    cw_bf = np.asarray(conv_w, dtype=np.float32).astype(BF16)
    return [
        {"cw": np.ascontiguousarray(cw_bf[:, i * P:(i + 1) * P, :])}
        for i in range(NCORES)
    ]


def prep_l2_inputs(hidden_states, wt_full, norm_w):
    x_flat = np.asarray(hidden_states, dtype=np.float32).reshape(TOK, H)
    xt_bf = np.ascontiguousarray(x_flat.T.astype(BF16))   # [h_in, tok]
    nw = np.asarray(norm_w, dtype=np.float32)
    return [
        {
            "xt": np.ascontiguousarray(xt_bf[:, i * TPC:(i + 1) * TPC]),
            "wt": wt_full,
            "nw": nw,
        }
        for i in range(NCORES)
    ]


def assemble_wt(res1_results):
    """[128 h_out, 1024 h_in] pieces -> W_sum^T [h_in, h_out] (host transpose)."""
    return np.ascontiguousarray(
        np.concatenate([res1_results[i]["wtp"].T for i in range(NCORES)], axis=1)
    )


def kernel(hidden_states, conv_w, norm_w):
    in_dtype = hidden_states.dtype
    core_ids = list(range(NCORES))

    # Launch 1: weight prep
    nc1 = _get("l1", build_l1)
    res1 = bass_utils.run_bass_kernel_spmd(nc1, prep_l1_inputs(conv_w), core_ids)
    wt_full = assemble_wt(res1.results)

    # Launch 2: GEMM + norm over token shards
    nc2 = _get("l2", build_l2)
    in2 = prep_l2_inputs(hidden_states, wt_full, norm_w)
    res2 = bass_utils.run_bass_kernel_spmd(nc2, in2, core_ids)
    y = np.concatenate([res2.results[i]["y"] for i in range(NCORES)], axis=0)
    return y.astype(np.float32).reshape(B, S, H).astype(in_dtype, copy=False)


# revision 20
# speedup vs baseline: 1.0778x; 1.0778x over previous
"""Trainium2 Bass kernel for nn_Conv2DLinalgRMSNorm.

Math: out = RMSNormEps(x @ (sum_l conv_w[l])^T / 20) * norm_w
  where RMSNormEps(v) = v / sqrt(sum_h v^2 + eps*H) * sqrt(H)

Strategy (8 NeuronCores, no cross-core collectives), all GEMM I/O in bf16:
  Host prep (layout/dtype only): conv_w and x^T are cast to bf16; x is
  pre-transposed to [h_in, tok] so L2 needs no on-chip transposes.
  Launch 1 (weight prep, sharded over 128-row output-channel slices):
    core i reads conv_w[:, i*128:(i+1)*128, :] in bf16 (5.2 MB, 2 KiB DMA
    lines on 2 queues), accumulates 20 layers via two running-sum chains
    (evens on DVE, odds on GpSimd), PE transpose-accumulates the two
    chains per 128-block and writes its [1024, 128] slice of W_sum^T bf16.
  Launch 2 (token-parallel GEMM + norm):
    core i loads full W^T (2 MB) + its x^T token slice (2 MB), runs the
    [1024 tok x 1024 x 1024] GEMM with h_in-chunk-outer ordering over 4
    concurrent PSUM tiles (all 8 banks) so the PE streams while DMA fills,
    then fuses LinalgRMSNorm on ACT/DVE.  The 1/20 scaling folds into the
    rsqrt bias: out = y * 32 * rsqrt(sum y^2 + 400*eps*H) * norm_w.
"""
import numpy as np
import ml_dtypes

import concourse.bass as bass
import concourse.mybir as mybir
from concourse.tile import TileContext
from concourse import bass_utils

dt = mybir.dt
BF16 = ml_dtypes.bfloat16
P = 128
H = 1024
NL = 20
B, S = 2, 4096
TOK = B * S            # 8192
NCORES = 8
TPC = TOK // NCORES    # 1024 tokens per core
NCH = H // P           # 8 h_in chunks of 128
NT = TPC // P          # 8 token tiles per core
EPS = 1e-6
SSQ_BIAS = float(NL * NL * EPS * H)   # 0.4096

_ctr = [0]


def _legalize_waits(nc):
    """This walrus build accepts 1 sync wait per instruction (2 on
    EventSemaphore); split excess waits into standalone waits."""
    def fix_block(blk):
        insts = list(blk.instructions)
        out = []
        changed = False
        for inst in insts:
            si = inst.sync_info
            waits = list(si.on_wait) if si and si.on_wait else []
            cap = 2 if isinstance(inst, mybir.InstEventSemaphore) else 1
            if len(waits) > cap:
                changed = True
                keep = waits[:cap]
                extra = waits[cap:]
                for i in range(0, len(extra), 2):
                    chunk = extra[i:i + 2]
                    _ctr[0] += 1
                    ev = mybir.InstEventSemaphore(
                        name=f"I-waitfix-{_ctr[0]}",
                        engine=inst.engine,
                        ins=[],
                        outs=[],
                        sync_info=mybir.SyncInfo(on_wait=chunk, on_update=[]),
                    )
                    out.append(ev)
                si.on_wait = keep
            out.append(inst)
        if changed:
            blk.instructions = out
        for sub in getattr(blk, "blocks", None) or []:
            fix_block(sub)

    for fn in nc.m.functions:
        for blk in fn.blocks:
            fix_block(blk)


def _make_identity(nc, identity):
    nc.gpsimd.memset(identity, 0.0)
    nc.gpsimd.affine_select(
        out=identity,
        in_=identity,
        compare_op=mybir.AluOpType.not_equal,
        fill=1.0,
        base=0,
        pattern=[[-1, identity.shape[0]]],
        channel_multiplier=1,
    )


def build_l1():
    """Weight prep: conv slice [20, 128, 1024] bf16 -> piece [128, 1024] bf16.

    Pure DMA + DVE: per-layer contiguous loads (2 KiB lines) over three DMA
    queues feed two all-bf16 parity running-sum chains on DVE (2-byte
    operands hit the DVE 2x mode); a final add merges them and the piece
    streams out untransposed with 2 KiB lines.  No PE, no PSUM.
    """
    nc = bass.Bass('TRN2', target_bir_lowering=False, debug=False)
    cw = nc.dram_tensor("cw", [NL, P, H], dt.bfloat16, kind="ExternalInput")
    # untransposed piece [128 h_out rows, 1024 h_in] -- 2 KiB DMA lines;
    # the host transposes pieces for free when assembling L2's W^T
    wtp = nc.dram_tensor("wtp", [P, H], dt.bfloat16, kind="ExternalOutput")
    with TileContext(nc) as tc:
        with (
            tc.tile_pool(name="ld", bufs=1) as ld,
            tc.tile_pool(name="acc", bufs=1) as accp,
        ):
            t = ld.tile([P, NL, H], dt.bfloat16, tag="t")
            cwr = cw.rearrange("l p h -> p l h")
            # evens on sync, odds on scalar, last four on the slower
            # gpsimd SWDGE queue
            for k in range(8):
                nc.sync.dma_start(t[:, 2 * k, :], cwr[:, 2 * k, :])
                nc.scalar.dma_start(t[:, 2 * k + 1, :], cwr[:, 2 * k + 1, :])
            for l in range(16, NL):
                nc.gpsimd.dma_start(t[:, l, :], cwr[:, l, :])

            # all-bf16 parity running sums, both on DVE (2x 16-bit mode),
            # consuming in arrival order: the gpsimd layers (16-19) land
            # early, each queue's last load (14/15) lands last
            ev_order = [0, 2, 4, 6, 8, 10, 12, 16, 18, 14]
            od_order = [1, 3, 5, 7, 9, 11, 13, 17, 19, 15]
            se = [accp.tile([P, H], dt.bfloat16, tag=f"se{i}", name=f"se{i}") for i in range(2)]
            so = [accp.tile([P, H], dt.bfloat16, tag=f"so{i}", name=f"so{i}") for i in range(2)]
            nc.vector.tensor_add(se[0][:], t[:, ev_order[0], :], t[:, ev_order[1], :])
            nc.vector.tensor_add(so[0][:], t[:, od_order[0], :], t[:, od_order[1], :])
            ce = co = 0
            for k in range(2, NL // 2):
                nc.vector.tensor_add(se[1 - ce][:], se[ce][:], t[:, ev_order[k], :])
                ce = 1 - ce
                nc.vector.tensor_add(so[1 - co][:], so[co][:], t[:, od_order[k], :])
                co = 1 - co

            wb = accp.tile([P, H], dt.bfloat16, tag="wb")
            nc.vector.tensor_add(wb[:], se[ce][:], so[co][:])
            nc.sync.dma_start(wtp[:, :], wb[:])
    _legalize_waits(nc)
    return nc


def build_l2():
    """Token-shard GEMM + LinalgRMSNorm, all-bf16 I/O, no on-chip transposes.

    x^T slice [1024 (h_in), 1024 (tok)] and W^T [1024, 1024] stream in as 8
    h_in chunks each on separate queues; the GEMM runs h_in-chunk-outer over
    4 concurrent [128, 1024] PSUM tiles (2 groups of 4 token tiles) so the
    PE consumes chunks as they land.  Norm fuses on ACT/DVE per token tile.
    """
    nc = bass.Bass('TRN2', target_bir_lowering=False, debug=False)
    xt = nc.dram_tensor("xt", [H, TPC], dt.bfloat16, kind="ExternalInput")
    wt = nc.dram_tensor("wt", [H, H], dt.bfloat16, kind="ExternalInput")
    nw = nc.dram_tensor("nw", [H], dt.float32, kind="ExternalInput")
    y = nc.dram_tensor("y", [TPC, H], dt.bfloat16, kind="ExternalOutput")
    NG = 2               # psum groups
    GT = NT // NG        # 4 token tiles per group
    with TileContext(nc) as tc:
        with (
            tc.tile_pool(name="w", bufs=1) as wp,
            tc.tile_pool(name="sq", bufs=2) as sqp,
            tc.tile_pool(name="yout", bufs=3) as yp,
            tc.tile_pool(name="stat", bufs=4) as stat,
            tc.tile_pool(name="psum", bufs=1, space="PSUM") as psum,
        ):
            xt_sb = wp.tile([P, NCH, TPC], dt.bfloat16, tag="xt_sb")
            wt_sb = wp.tile([P, NCH, H], dt.bfloat16, tag="wt_sb")
            xt_r = xt.rearrange("(c p) t -> p c t", p=P)
            wt_r = wt.rearrange("(c p) o -> p c o", p=P)
            # identity first so the PE warm-up unblocks early; gpsimd DMA
            # issues follow it in that engine's program order
            ident = wp.tile([P, P], dt.float32, tag="ident")
            _make_identity(nc, ident[:])
            # paired interleaved fill: chunk pair (xt_k, wt_k) completes at
            # ~0.73us intervals across the two HWDGE queues, always ahead
            # of the GEMM wave that consumes it
            for hc in range(NCH):
                if hc % 2 == 0:
                    nc.sync.dma_start(xt_sb[:, hc, :], xt_r[:, hc, :])
                    nc.scalar.dma_start(wt_sb[:, hc, :], wt_r[:, hc, :])
                else:
                    nc.sync.dma_start(wt_sb[:, hc, :], wt_r[:, hc, :])
                    nc.scalar.dma_start(xt_sb[:, hc, :], xt_r[:, hc, :])
            nwb = wp.tile([P, H], dt.bfloat16, tag="nwb")
            nc.gpsimd.dma_start(nwb[:], nw[None, :].partition_broadcast(P))

            pts = [psum.tile([P, H], dt.float32, tag=f"pt{i}", name=f"pt{i}") for i in range(GT)]
            # PE warm-up into a psum half that hc=0 later resets (start=True)
            for _ in range(12):
                nc.tensor.matmul(pts[0][:, bass.ds(0, P)], ident[:], ident[:],
                                 is_transpose=True, start=True, stop=True)

            def norm_out(tt, pt, release=False):
                if release:
                    # ACT copy releases the psum tile fast so the next GEMM
                    # tile can reuse it; the norm then runs from SBUF
                    src = sqp.tile([P, H], dt.bfloat16, tag="ptc", name="ptc")
                    nc.scalar.activation(
                        src[:], pt[:], mybir.ActivationFunctionType.Copy,
                    )
                else:
                    src = pt
                sq = sqp.tile([P, H], dt.bfloat16, tag="sq", name="sq")
                v = stat.tile([P, 1], dt.float32, tag="v", name="v")
                nc.scalar.activation(
                    sq[:], src[:], mybir.ActivationFunctionType.Square,
                    accum_out=v[:],
                )
                vb = stat.tile([P, 1], dt.float32, tag="vb", name="vb")
                nc.vector.tensor_scalar(
                    vb[:], v[:], SSQ_BIAS, None, mybir.AluOpType.add,
                )
                rv = stat.tile([P, 1], dt.float32, tag="rv", name="rv")
                nc.vector.reciprocal(rv[:], vb[:])
                s = stat.tile([P, 1], dt.float32, tag="s", name="s")
                nc.scalar.activation(
                    s[:], rv[:], mybir.ActivationFunctionType.Sqrt,
                    scale=float(H),
                )
                ysb = yp.tile([P, H], dt.bfloat16, tag="ysb", name="ysb")
                nc.vector.scalar_tensor_tensor(
                    ysb[:], src[:], s[:], nwb[:],
                    op0=mybir.AluOpType.mult, op1=mybir.AluOpType.mult,
                )
                nc.sync.dma_start(y[bass.ds(tt * P, P), :], ysb[:])

            # group 0 (tiles 0-3): hc-outer waves track the DMA fill
            for hc in range(NCH):
                for i in range(GT):
                    lhs = xt_sb[:, hc, bass.ds(i * P, P)]
                    for oh in range(2):
                        osl = bass.ds(oh * 512, 512)
                        nc.tensor.matmul(
                            pts[i][:, osl], lhs, wt_sb[:, hc, osl],
                            start=(hc == 0), stop=(hc == NCH - 1),
                        )
            for i in range(GT):
                norm_out(i, pts[i], release=True)
            # tiles 4-7 sequential on slots 0-3: each slot's psum is
            # released ~1us after its stop by the ACT copy
            for i in range(GT):
                tt = GT + i
                for hc in range(NCH):
                    lhs = xt_sb[:, hc, bass.ds(tt * P, P)]
                    for oh in range(2):
                        osl = bass.ds(oh * 512, 512)
                        nc.tensor.matmul(
                            pts[i][:, osl], lhs, wt_sb[:, hc, osl],
                            start=(hc == 0), stop=(hc == NCH - 1),
                        )
                norm_out(tt, pts[i])
    _legalize_waits(nc)
    return nc


_CACHE = {}


def _get(name, builder):
    if name not in _CACHE:
        _CACHE[name] = builder()
    return _CACHE[name]


def prep_l1_inputs(conv_w):
    cw_bf = np.asarray(conv_w, dtype=np.float32).astype(BF16)
    return [
        {"cw": np.ascontiguousarray(cw_bf[:, i * P:(i + 1) * P, :])}
        for i in range(NCORES)
    ]


def prep_l2_inputs(hidden_states, wt_full, norm_w):
    x_flat = np.asarray(hidden_states, dtype=np.float32).reshape(TOK, H)
    xt_bf = np.ascontiguousarray(x_flat.T.astype(BF16))   # [h_in, tok]
    nw = np.asarray(norm_w, dtype=np.float32)
    return [
        {
            "xt": np.ascontiguousarray(xt_bf[:, i * TPC:(i + 1) * TPC]),
            "wt": wt_full,
            "nw": nw,
        }
        for i in range(NCORES)
    ]


def assemble_wt(res1_results):
    """[128 h_out, 1024 h_in] pieces -> W_sum^T [h_in, h_out] (host transpose)."""
    return np.ascontiguousarray(
        np.concatenate([res1_results[i]["wtp"].T for i in range(NCORES)], axis=1)
    )


def kernel(hidden_states, conv_w, norm_w):
    in_dtype = hidden_states.dtype
    core_ids = list(range(NCORES))

    # Launch 1: weight prep
    nc1 = _get("l1", build_l1)
    res1 = bass_utils.run_bass_kernel_spmd(nc1, prep_l1_inputs(conv_w), core_ids)
    wt_full = assemble_wt(res1.results)

    # Launch 2: GEMM + norm over token shards
    nc2 = _get("l2", build_l2)
    in2 = prep_l2_inputs(hidden_states, wt_full, norm_w)
    res2 = bass_utils.run_bass_kernel_spmd(nc2, in2, core_ids)
    y = np.concatenate([res2.results[i]["y"] for i in range(NCORES)], axis=0)
    return y.astype(np.float32).reshape(B, S, H).astype(in_dtype, copy=False)
